# revision 14
# baseline (speedup 1.0000x reference)
"""Distributed Trainium2 kernel for nn_AMKPDModel_19902878450348.

Full model on device. Sharding: tokens (B*N = 2048) split 8 ways, 256 active
tokens per core (+8-token halo each side for depthwise-conv erosion over the
8 blocks). Cores 0-3 hold batch 0, cores 4-7 batch 1. Per block, each core
computes qkv/rope/rms for its shard, AllGathers k-hat/v (bf16) across its
4-core batch group, runs full attention for its query rows, then the MLP +
depthwise conv token-locally. lm_head: final hidden is AllGathered across all
8 cores, each core computes a 4000-column vocab shard of the logits.

rotate_half is folded into the QKV weights host-side (extra matmul columns);
RMS scale factors are computed pre-rotation (rotation preserves per-head
norms) via selector matmuls + exp(-0.5*ln(x)) on ScalarE; the attention
weight (elu(s)+1)^2 is computed as (max(s+1, min(e^s, 1)))^2.
"""

import numpy as np

D = 512; H = 8; DH = 64; L = 4; INNER = 1536; V = 32000
HCYC = 2; KS = 3
NCORES = 8
B = 2; N = 1024; TOK = 2048
KO = D // 128            # 4 contraction tiles
SH = 256                 # active tokens per core
HALO = 8
W = SH + 2 * HALO        # 272 working width
A0, A1 = HALO, HALO + SH # active col range [8, 264)
NB = HCYC * L            # 8 blocks
KT = N // 128            # 8 k-token tiles
MTS = [128, 128, 16]     # token M-tile sizes covering W
MLPC = INNER // 128      # 12
VSH = V // NCORES        # 4000 vocab shard
VC = 8; VCW = VSH // VC  # 8 vocab chunks of 500
NEG_QBIAS = float(-0.25 * np.log(DH))  # folds DH^-0.5 into both rms scales

LAST_EXEC_NS = None
DEBUG = False

_CACHED = {}


# ---------------------------------------------------------------------------
# device kernel build
# ---------------------------------------------------------------------------

def _build_nc(debug=False, silu_sigmoid=False):
    import concourse.bass as bass  # noqa: F401
    import concourse.mybir as mybir
    import concourse.tile as tile
    from concourse import bacc

    f32 = mybir.dt.float32
    f32r = mybir.dt.float32r
    bf16 = mybir.dt.bfloat16
    AF = mybir.ActivationFunctionType
    OP = mybir.AluOpType

    nc = bacc.Bacc("TRN2", target_bir_lowering=False, debug=False,
                   num_devices=NCORES)

    # ---- external inputs (per core) ----
    hT0_d = nc.dram_tensor("hT0", [128, KO, W], f32r, kind="ExternalInput")
    x_d = nc.dram_tensor("x", [128, KO, W], f32r, kind="ExternalInput")
    cos_d = nc.dram_tensor("cosb", [128, W], f32, kind="ExternalInput")
    sin_d = nc.dram_tensor("sinb", [128, W], f32, kind="ExternalInput")
    cmi_d = nc.dram_tensor("cmi", [128, 2], mybir.dt.uint8, kind="ExternalInput")
    wqkv_d = nc.dram_tensor("wqkv", [L, 128, KO, 2560], f32r, kind="ExternalInput")
    wo_d = nc.dram_tensor("wo", [L, 128, KO, 512], f32r, kind="ExternalInput")
    wup_d = nc.dram_tensor("wup", [L, 128, KO, 3072], f32r, kind="ExternalInput")
    wdn_d = nc.dram_tensor("wdn", [L, 128, MLPC, 512], f32r, kind="ExternalInput")
    cw_d = nc.dram_tensor("cw", [128, L * MLPC, KS], f32, kind="ExternalInput")
    cb_d = nc.dram_tensor("cb", [128, L * MLPC], f32, kind="ExternalInput")
    lm_d = nc.dram_tensor("lm", [128, KO, VSH], f32r, kind="ExternalInput")
    sel2_d = nc.dram_tensor("sel2", [2, 128], f32r, kind="ExternalInput")
    idr_d = nc.dram_tensor("idr", [128, 128], f32r, kind="ExternalInput")
    conf_d = nc.dram_tensor("conf", [128, 3], f32, kind="ExternalInput")
    conr_d = nc.dram_tensor("conr", [128, 2], f32r, kind="ExternalInput")
    onr1_d = nc.dram_tensor("onr1", [1, 128], f32r, kind="ExternalInput")
    onb_d = nc.dram_tensor("onb", [128, 8], mybir.dt.bfloat16, kind="ExternalInput")
    idb_d = nc.dram_tensor("idb", [128, 128], mybir.dt.bfloat16, kind="ExternalInput")
    sel2T_d = nc.dram_tensor("sel2T", [128, 2], f32r, kind="ExternalInput")

    logits_d = nc.dram_tensor("logits", [TOK, VSH], f32, kind="ExternalOutput")
    if debug:
        dbg_d = nc.dram_tensor("dbg", [NB, KO, 128, SH], f32r, kind="ExternalOutput")

    RG_KV = [[0, 1, 2, 3], [4, 5, 6, 7]]
    RG_ALL = [[0, 1, 2, 3, 4, 5, 6, 7]]

    def r_(ap):
        return ap.bitcast(f32r)

    with tile.TileContext(nc) as tc:
      with tc.tile_pool(name="pers", bufs=1) as pers, \
           tc.tile_pool(name="hTp", bufs=3) as hTp, \
           tc.tile_pool(name="wq1", bufs=2) as wq1, \
           tc.tile_pool(name="wbig", bufs=1) as wbig, \
           tc.tile_pool(name="wmlp", bufs=2) as wmlp, \
           tc.tile_pool(name="wlm", bufs=2) as wlm, \
           tc.tile_pool(name="act", bufs=1) as act, \
           tc.tile_pool(name="act2", bufs=2) as act2, \
           tc.tile_pool(name="dpool", bufs=2, space="DRAM") as dpool, \
           tc.tile_pool(name="p1", bufs=6, space="PSUM") as p1, \
           tc.tile_pool(name="p2", bufs=1, space="PSUM") as p2:

        # ---- persistent constants ----
        X = pers.tile([128, KO, W], f32r, tag="X")
        nc.sync.dma_start(X[:], x_d[:])
        cosb = pers.tile([128, W], f32, tag="cosb")
        nc.sync.dma_start(cosb[:], cos_d[:])
        sinb = pers.tile([128, W], f32, tag="sinb")
        nc.sync.dma_start(sinb[:], sin_d[:])
        cmi = pers.tile([128, 2], mybir.dt.uint8, tag="cmi")
        nc.sync.dma_start(cmi[:], cmi_d[:])
        cw = pers.tile([128, L * MLPC, KS], f32, tag="cw")
        nc.sync.dma_start(cw[:], cw_d[:])
        cb = pers.tile([128, L * MLPC], f32, tag="cb")
        nc.sync.dma_start(cb[:], cb_d[:])

        identR = pers.tile([128, 128], f32r, tag="identR")
        nc.sync.dma_start(identR[:], idr_d[:])
        identB = pers.tile([128, 128], bf16, tag="identB")
        nc.sync.dma_start(identB[:], idb_d[:])

        sel2 = pers.tile([2, 128], f32r, tag="sel2")
        nc.sync.dma_start(sel2[:], sel2_d[:])
        sel2T = pers.tile([128, 2], f32r, tag="sel2T")
        nc.sync.dma_start(sel2T[:], sel2T_d[:])
        conf = pers.tile([128, 3], f32, tag="conf")
        nc.sync.dma_start(conf[:], conf_d[:])
        epsQ = conf[:, 0:1]
        bqk = conf[:, 1:2]
        zer = conf[:, 2:3]
        conr = pers.tile([128, 2], f32r, tag="conr")
        nc.sync.dma_start(conr[:], conr_d[:])
        onesK = conr[:, 0:1]
        zerR = conr[:, 1:2]
        ones1 = pers.tile([1, 128], f32r, tag="ones1")
        nc.sync.dma_start(ones1[:], onr1_d[:])
        onesB = pers.tile([128, 8], bf16, tag="onesB")
        nc.sync.dma_start(onesB[:], onb_d[:])

        hT = hTp.tile([128, KO, W], f32r, tag="hT")
        nc.sync.dma_start(hT[:], hT0_d[:])

        # -------------------------------------------------------------------
        def mt_slices():
            out = []
            off = 0
            for mw in MTS:
                out.append((off, mw))
                off += mw
            return out

        MSL = mt_slices()

        def emit_rmsD(src, dst):
            """dst = rms_norm(src) over D (both [128, KO, W] f32)."""
            ps_ssq = p1.tile([1, W], f32, tag="p1")
            for ft in range(KO):
                sqd = act2.tile([128, W], f32r, tag="sqd")
                nc.scalar.activation(out=sqd[:], in_=src[:, ft, :], func=AF.Square)
                nc.tensor.matmul(ps_ssq[:], onesK, r_(sqd[:]),
                                 start=(ft == 0), stop=(ft == KO - 1))
            ln1 = act2.tile([1, W], f32, tag="ln1")
            nc.scalar.activation(out=ln1[:], in_=ps_ssq[:], func=AF.Ln,
                                 scale=1.0 / D, bias=epsQ[0:1])
            rstd1 = act2.tile([1, W], f32r, tag="rstd1")
            nc.scalar.activation(out=rstd1[:], in_=ln1[:], func=AF.Exp, scale=-0.5)
            ps_bc = p1.tile([128, W], f32, tag="p1")
            nc.tensor.matmul(ps_bc[:], ones1, r_(rstd1[:]),
                             start=True, stop=True)
            for ft in range(KO):
                nc.vector.tensor_mul(dst[:, ft, :], src[:, ft, :], ps_bc[:])

        # -------------------------------------------------------------------
        def emit_block(bi):
            nonlocal hT
            li = bi % L

            wv = wbig.tile([128, KO, 512], f32r, tag="wv")
            nc.sync.dma_start(wv[:], wqkv_d[li, :, :, 1024:1536])
            wo_t = wbig.tile([128, KO, 512], f32r, tag="wo")
            nc.sync.dma_start(wo_t[:], wo_d[li])

            kvi = dpool.tile([8, 128, SH], bf16, tag="kvin")

            # ---- Q/K projections + rope + rms ----
            qhat = act.tile([128, KO, W], bf16, tag="qhat")
            khat = act.tile([128, KO, W], bf16, tag="khat")
            for grp in range(2):
                base = 512 * grp
                rbase = 1536 + 512 * grp
                for ft in range(KO):
                    wsl = wq1.tile([128, KO, 128], f32r, tag="wqsl")
                    nc.sync.dma_start(
                        wsl[:], wqkv_d[li, :, :, base + 128 * ft: base + 128 * ft + 128])
                    wrl = wq1.tile([128, KO, 128], f32r, tag="wrsl")
                    nc.sync.dma_start(
                        wrl[:], wqkv_d[li, :, :, rbase + 128 * ft: rbase + 128 * ft + 128])
                    ps_q = p1.tile([128, W], f32, tag="p1")
                    for k in range(KO):
                        nc.tensor.matmul(ps_q[:], r_(wsl[:, k, :]), r_(hT[:, k, :]),
                                         start=(k == 0), stop=(k == KO - 1))
                    ps_r = p1.tile([128, W], f32, tag="p1")
                    for k in range(KO):
                        nc.tensor.matmul(ps_r[:], r_(wrl[:, k, :]), r_(hT[:, k, :]),
                                         start=(k == 0), stop=(k == KO - 1))
                    sq = act2.tile([128, W], f32r, tag="sq")
                    nc.scalar.activation(out=sq[:], in_=ps_q[:], func=AF.Square)
                    ps_ssq = p1.tile([2, W], f32, tag="p1")
                    nc.tensor.matmul(ps_ssq[:], r_(sel2T[:]), r_(sq[:]),
                                     start=True, stop=True)
                    lnv = act2.tile([2, W], f32, tag="lnv")
                    nc.scalar.activation(out=lnv[:], in_=ps_ssq[:], func=AF.Ln,
                                         scale=1.0 / DH, bias=epsQ[0:2])
                    rstd = act2.tile([2, W], f32r, tag="rstd")
                    nc.scalar.activation(out=rstd[:], in_=lnv[:], func=AF.Exp,
                                         scale=-0.5, bias=bqk[0:2])
                    ps_bc = p1.tile([128, W], f32, tag="p1")
                    nc.tensor.matmul(ps_bc[:], r_(sel2[:]), r_(rstd[:]),
                                     start=True, stop=True)
                    t1 = act2.tile([128, W], f32, tag="t1")
                    nc.vector.tensor_mul(t1[:], ps_q[:], cosb[:])
                    t2 = act2.tile([128, W], f32, tag="t2")
                    nc.vector.tensor_mul(t2[:], ps_r[:], sinb[:])
                    nc.vector.tensor_add(t1[:], t1[:], t2[:])
                    if grp == 0:
                        nc.vector.tensor_mul(qhat[:, ft, :], t1[:], ps_bc[:])
                    else:
                        nc.vector.tensor_mul(khat[:, ft, :], t1[:], ps_bc[:])
                        nc.sync.dma_start(kvi[ft], khat[:, ft, A0:A1])

            # ---- vT (bf16, for gather) ----
            vT = act.tile([128, KO, W], bf16, tag="vT")
            for ft in range(KO):
                ps_v = p1.tile([128, W], f32, tag="p1")
                for k in range(KO):
                    nc.tensor.matmul(ps_v[:], r_(wv[:, k, 128 * ft:128 * ft + 128]),
                                     r_(hT[:, k, :]),
                                     start=(k == 0), stop=(k == KO - 1))
                nc.scalar.activation(out=vT[:, ft, :], in_=ps_v[:], func=AF.Copy)
                nc.sync.dma_start(kvi[4 + ft], vT[:, ft, A0:A1])

            # ---- v_loc (layout A [tok, feat], f32) ----
            vloc = act.tile([128, len(MTS), 512], f32, tag="vloc")
            for mt, (off, mw) in enumerate(MSL):
                ps_v = p1.tile([128, 512], f32, tag="p1")
                for k in range(KO):
                    nc.tensor.matmul(ps_v[0:mw, :], r_(hT[:, k, off:off + mw]),
                                     r_(wv[:, k, :]),
                                     start=(k == 0), stop=(k == KO - 1))
                nc.scalar.activation(out=vloc[0:mw, mt, :], in_=ps_v[0:mw, :],
                                     func=AF.Copy)

            # ---- AllGather k-hat + vT across batch group ----
            kvo = dpool.tile([4, 8, 128, SH], bf16, tag="kvout")
            nc.gpsimd.collective_compute(
                "AllGather", OP.bypass,
                ins=[kvi.opt()], outs=[kvo.opt()],
                replica_groups=RG_KV)

            kf = act.tile([128, KO, 4, SH], bf16, tag="kf")
            vf = act.tile([128, KO, 4, SH], bf16, tag="vf")
            for ft in range(KO):
                for g in range(4):
                    nc.sync.dma_start(kf[:, ft, g], kvo[g, ft])
                    nc.sync.dma_start(vf[:, ft, g], kvo[g, 4 + ft])

            # ---- v_aug: transpose vf to [tok, feat] blocks + ones column ----
            vaug = act.tile([128, KT, H, 65], bf16, tag="vaug")
            for tt in range(KT):
                g4, toff = tt // 2, (tt % 2) * 128
                for ft in range(KO):
                    psv = p1.tile([128, 128], bf16, tag="p1")
                    nc.tensor.transpose(psv[:], vf[:, ft, g4, toff:toff + 128],
                                        identB[:])
                    nc.vector.tensor_copy(
                        out=vaug[:, tt, 2 * ft:2 * ft + 2, 0:64],
                        in_=psv[:].rearrange("p (h d) -> p h d", h=2))
                nc.vector.tensor_copy(out=vaug[:, tt, :, 64:65],
                                      in_=onesB[:, :].rearrange('p (h o) -> p h o', o=1))

            # ---- attention per head ----
            m = act.tile([128, len(MTS), 512], f32r, tag="m")
            for h in range(H):
                po = 64 * (h % 2)
                ftq = h // 2
                Wt = act2.tile([128, KT, W], bf16, tag="Wt")
                for g in range(KT // 2):
                    ps_st = p2.tile([128, 2, 512], f32, tag="p2")
                    for i in range(2):
                        kt = 2 * g + i
                        nc.tensor.matmul(
                            ps_st[:, i, 0:W],
                            kf[po:po + 64, ftq, kt // 2,
                               (kt % 2) * 128:(kt % 2) * 128 + 128],
                            qhat[po:po + 64, ftq, :],
                            start=True, stop=True)
                    ev = act2.tile([128, 2, W], bf16, tag="ev")
                    nc.scalar.activation(out=ev[:], in_=ps_st[:, :, 0:W],
                                         func=AF.Exp)
                    tmin = act2.tile([128, 2, W], bf16, tag="tmin")
                    nc.vector.tensor_scalar_min(tmin[:], ev[:], 1.0)
                    wmax = act2.tile([128, 2, W], bf16, tag="wmax")
                    nc.vector.scalar_tensor_tensor(
                        out=wmax[:], in0=ps_st[:, :, 0:W], scalar=1.0,
                        in1=tmin[:], op0=OP.add, op1=OP.max)
                    nc.gpsimd.tensor_mul(Wt[:, 2 * g:2 * g + 2, :], wmax[:], wmax[:])
                for mt, (off, mw) in enumerate(MSL):
                    ps_at = p1.tile([128, 128], f32, tag="p1")
                    for kt in range(KT):
                        nc.tensor.matmul(ps_at[0:mw, 0:65],
                                         Wt[:, kt, off:off + mw],
                                         vaug[:, kt, h, :],
                                         start=(kt == 0), stop=(kt == KT - 1))
                    rc = act2.tile([128, 1], f32, tag="rc")
                    nc.vector.reciprocal(rc[0:mw, :], ps_at[0:mw, 64:65])
                    nc.vector.scalar_tensor_tensor(
                        out=m[0:mw, mt, 64 * h:64 * h + 64],
                        in0=ps_at[0:mw, 0:64], scalar=rc[0:mw, 0:1],
                        in1=vloc[0:mw, mt, 64 * h:64 * h + 64],
                        op0=OP.mult, op1=OP.subtract)

            # ---- mT + Wo + residual + rms ----
            mT = act.tile([128, KO, W], f32r, tag="mT")
            for ft in range(KO):
                ps_mt = p1.tile([128, W], f32r, tag="p1")
                for mt, (off, mw) in enumerate(MSL):
                    nc.tensor.transpose(ps_mt[:, off:off + mw],
                                        m[0:mw, mt, 128 * ft:128 * ft + 128],
                                        identR[0:mw, 0:mw])
                nc.scalar.activation(out=mT[:, ft, :], in_=ps_mt[:], func=AF.Copy)

            h2p = act.tile([128, KO, W], f32, tag="h2p")
            for mt in range(KO):
                ps_o = p1.tile([128, W], f32, tag="p1")
                for k in range(KO):
                    nc.tensor.matmul(ps_o[:], r_(wo_t[:, k, 128 * mt:128 * mt + 128]),
                                     r_(mT[:, k, :]),
                                     start=(k == 0), stop=(k == KO - 1))
                nc.vector.tensor_add(h2p[:, mt, :], hT[:, mt, :], ps_o[:])
            hmid = hTp.tile([128, KO, W], f32r, tag="hT")
            emit_rmsD(h2p, hmid)

            # ---- MLP + depthwise conv ----
            ps_dn = [p1.tile([128, W], f32, tag="p1", name=f"psdn{mt}")
                     for mt in range(KO)]
            for c in range(MLPC):
                wgsl = wmlp.tile([128, KO, 128], f32r, tag="wupg")
                nc.sync.dma_start(wgsl[:], wup_d[li, :, :, 128 * c:128 * c + 128])
                wusl = wmlp.tile([128, KO, 128], f32r, tag="wupu")
                nc.sync.dma_start(
                    wusl[:], wup_d[li, :, :, INNER + 128 * c:INNER + 128 * c + 128])
                wdsl = wmlp.tile([128, 512], f32r, tag="wdn")
                nc.sync.dma_start(wdsl[:], wdn_d[li, :, c, :])

                ps_g = p1.tile([128, W], f32, tag="p1")
                for k in range(KO):
                    nc.tensor.matmul(ps_g[:], r_(wgsl[:, k, :]), r_(hmid[:, k, :]),
                                     start=(k == 0), stop=(k == KO - 1))
                gs = act2.tile([128, W], f32, tag="gs")
                if silu_sigmoid:
                    nc.scalar.activation(out=gs[:], in_=ps_g[:], func=AF.Sigmoid)
                    nc.vector.tensor_mul(gs[:], gs[:], ps_g[:])
                else:
                    nc.scalar.activation(out=gs[:], in_=ps_g[:], func=AF.Silu)
                ps_u = p1.tile([128, W], f32, tag="p1")
                for k in range(KO):
                    nc.tensor.matmul(ps_u[:], r_(wusl[:, k, :]), r_(hmid[:, k, :]),
                                     start=(k == 0), stop=(k == KO - 1))
                hf = act2.tile([128, W], f32, tag="hf")
                nc.vector.tensor_mul(hf[:], gs[:], ps_u[:])
                # zero out-of-batch columns adjacent to batch edges (NaN-safe)
                nc.vector.copy_predicated(hf[:, A0 - 1:A0], cmi[:, 0:1],
                                          zer)
                nc.vector.copy_predicated(hf[:, A1:A1 + 1], cmi[:, 1:2],
                                          zer)
                ci = MLPC * li + c
                ta = act2.tile([128, W - 2], f32, tag="ta")
                nc.scalar.activation(out=ta[:], in_=hf[:, 0:W - 2], func=AF.Copy,
                                     scale=cw[:, ci, 0:1])
                tb = act2.tile([128, W - 2], f32, tag="tb")
                nc.vector.scalar_tensor_tensor(
                    out=tb[:], in0=hf[:, 1:W - 1], scalar=cw[:, ci, 1:2],
                    in1=ta[:], op0=OP.mult, op1=OP.add)
                nc.vector.scalar_tensor_tensor(
                    out=ta[:], in0=hf[:, 2:W], scalar=cw[:, ci, 2:3],
                    in1=tb[:], op0=OP.mult, op1=OP.add)
                hc = act2.tile([128, W], f32r, tag="hc")
                nc.vector.tensor_copy(out=hc[:, 0:1], in_=zerR)
                nc.vector.tensor_copy(out=hc[:, W - 1:W], in_=zerR)
                if silu_sigmoid:
                    sgc = act2.tile([128, W - 2], f32, tag="sgc")
                    nc.scalar.activation(out=sgc[:], in_=ta[:], func=AF.Sigmoid,
                                         bias=cb[:, ci:ci + 1])
                    nc.vector.scalar_tensor_tensor(
                        out=hc[:, 1:W - 1], in0=ta[:], scalar=cb[:, ci:ci + 1],
                        in1=sgc[:], op0=OP.add, op1=OP.mult)
                else:
                    nc.scalar.activation(out=hc[:, 1:W - 1], in_=ta[:], func=AF.Silu,
                                         bias=cb[:, ci:ci + 1])
                for mt in range(KO):
                    nc.tensor.matmul(ps_dn[mt][:],
                                     r_(wdsl[:, 128 * mt:128 * mt + 128]),
                                     r_(hc[:]),
                                     start=(c == 0), stop=(c == MLPC - 1))

            h3p = act.tile([128, KO, W], f32, tag="h3p")
            for mt in range(KO):
                nc.vector.tensor_add(h3p[:, mt, :], hmid[:, mt, :], ps_dn[mt][:])
            hnext = hTp.tile([128, KO, W], f32r, tag="hT")
            emit_rmsD(h3p, hnext)
            if debug:
                for ft in range(KO):
                    nc.sync.dma_start(dbg_d[bi, ft], hnext[:, ft, A0:A1])
            if bi == L - 1:  # start of second cycle: hidden += X
                for ft in range(KO):
                    nc.vector.tensor_add(hnext[:, ft, :], hnext[:, ft, :],
                                         X[:, ft, :])
            hT = hnext

        for bi in range(NB):
            emit_block(bi)

        # ---- lm_head: AllGather final hidden, vocab-sharded matmul ----
        hid_in = dpool.tile([KO, 128, SH], f32r, tag="hidin")
        for ft in range(KO):
            nc.sync.dma_start(hid_in[ft], hT[:, ft, A0:A1])
        hid_out = dpool.tile([NCORES, KO, 128, SH], f32r, tag="hidout", addr_space="Shared")
        nc.gpsimd.collective_compute(
            "AllGather", OP.bypass,
            ins=[hid_in.opt()], outs=[hid_out.opt()],
            replica_groups=RG_ALL)

        for vc in range(VC):
            lsl = wlm.tile([128, KO, VCW], f32r, tag="lm")
            nc.sync.dma_start(lsl[:], lm_d[:, :, VCW * vc:VCW * vc + VCW])
            for mt in range(TOK // 128):
                hsl = wlm.tile([128, KO, 128], f32r, tag="hsl")
                g, toff = mt // 2, (mt % 2) * 128
                for k in range(KO):
                    nc.sync.dma_start(hsl[:, k, :],
                                      hid_out[g, k, :, toff:toff + 128])
                ps = p1.tile([128, VCW], f32, tag="p1")
                for k in range(KO):
                    nc.tensor.matmul(ps[:], r_(hsl[:, k, :]), r_(lsl[:, k, :]),
                                     start=(k == 0), stop=(k == KO - 1))
                ob = act2.tile([128, VCW], f32, tag="ob")
                if mt % 2 == 0:
                    nc.vector.tensor_copy(out=ob[:], in_=ps[:])
                else:
                    nc.scalar.activation(out=ob[:], in_=ps[:], func=AF.Copy)
                nc.sync.dma_start(
                    logits_d[128 * mt:128 * mt + 128, VCW * vc:VCW * vc + VCW],
                    ob[:])

    nc.compile()
    return nc


# ---------------------------------------------------------------------------
# host-side prep
# ---------------------------------------------------------------------------

def _rope_cache(n):
    inv = 1.0 / (10000.0 ** (np.arange(0, DH, 2, dtype=np.float32) / DH))
    t = np.arange(n, dtype=np.float32)
    fr = np.outer(t, inv).astype(np.float32)
    emb = np.concatenate([fr, fr], axis=-1)
    return np.cos(emb).astype(np.float32), np.sin(emb).astype(np.float32)


def _to_tiles(mat):
    """[D, F] -> [128, D//128, F] (partition-major view for lhsT loads)."""
    d, f = mat.shape
    return np.ascontiguousarray(
        mat.reshape(d // 128, 128, f).transpose(1, 0, 2))


def _prep_shared(Wqkv, Wo, Wup, conv_w, conv_b, Wdown, lm_head):
    """Weight tensors shared by all cores, pre-transposed/tiled."""
    wqkv_all = np.empty((L, 128, KO, 2560), np.float32)
    wo_all = np.empty((L, 128, KO, 512), np.float32)
    wup_all = np.empty((L, 128, KO, 3072), np.float32)
    wdn_all = np.empty((L, 128, MLPC, 512), np.float32)
    for li in range(L):
        wq = Wqkv[li].astype(np.float32)          # [1536, 512] rows q|k|v
        qT = wq[0:512].T                          # [512, 512] cols = q feats
        kT = wq[512:1024].T
        vT_ = wq[1024:1536].T
        # rotate_half folded: rot feature (h*64+i) = -feat(h*64+i+32) (i<32)
        #                                            +feat(h*64+i-32) (i>=32)
        def rotcols(wT):
            r = np.empty_like(wT)
            w4 = wT.reshape(512, H, DH)
            r4 = r.reshape(512, H, DH)
            r4[:, :, 0:32] = -w4[:, :, 32:64]
            r4[:, :, 32:64] = w4[:, :, 0:32]
            return r
        full = np.concatenate([qT, kT, vT_, rotcols(qT), rotcols(kT)], axis=1)
        wqkv_all[li] = _to_tiles(full)
        wo_all[li] = _to_tiles(Wo[li].astype(np.float32).T)
        wup_all[li] = _to_tiles(Wup[li].astype(np.float32).T)
        wdnT = Wdown[li].astype(np.float32).T     # [1536, 512]
        wdn_all[li] = np.ascontiguousarray(
            wdnT.reshape(MLPC, 128, 512).transpose(1, 0, 2))
    cw_all = np.ascontiguousarray(
        conv_w[:, :, 0, :].astype(np.float32)
        .reshape(L, MLPC, 128, KS).transpose(2, 0, 1, 3)
        .reshape(128, L * MLPC, KS))
    cb_all = np.ascontiguousarray(
        conv_b.astype(np.float32).reshape(L, MLPC, 128).transpose(2, 0, 1)
        .reshape(128, L * MLPC))
    lmT = lm_head.astype(np.float32).T            # [512, 32000]
    lm_shards = [
        _to_tiles(np.ascontiguousarray(lmT[:, c * VSH:(c + 1) * VSH]))
        for c in range(NCORES)]
    return wqkv_all, wo_all, wup_all, wdn_all, cw_all, cb_all, lm_shards


def _prep_in_maps(input_ids, embedding, init_hidden, shared):
    wqkv_all, wo_all, wup_all, wdn_all, cw_all, cb_all, lm_shards = shared
    ids = np.asarray(input_ids)
    Xf = (np.asarray(embedding, np.float32)[ids]
          * np.float32(np.sqrt(D)))                 # [B, N, D]
    init = np.asarray(init_hidden, np.float32)
    cos_full, sin_full = _rope_cache(N)             # [N, DH]

    import ml_dtypes
    ident_f32 = np.eye(128, dtype=np.float32)
    ident_bf16 = np.eye(128, dtype=ml_dtypes.bfloat16)
    conf_host = np.zeros((128, 3), np.float32)
    conf_host[:, 0] = 1e-5
    conf_host[:, 1] = NEG_QBIAS
    conr_host = np.zeros((128, 2), np.float32)
    conr_host[:, 0] = 1.0
    onr1_host = np.ones((1, 128), np.float32)
    onb_host = np.ones((128, 8), ml_dtypes.bfloat16)
    sel2_host = np.zeros((2, 128), np.float32)
    sel2_host[0, 0:64] = 1.0
    sel2_host[1, 64:128] = 1.0
    sel2T_host = np.ascontiguousarray(sel2_host.T)

    in_maps = []
    for c in range(NCORES):
        b, s = c // 4, c % 4
        g0 = s * SH - HALO                          # global start (may be <0)
        cols = np.arange(g0, g0 + W)
        valid = (cols >= 0) & (cols < N)
        colsc = np.clip(cols, 0, N - 1)

        xT = Xf[b].T[:, colsc].copy()               # [512, W]
        xT[:, ~valid] = 0.0
        h0 = xT + init[:, None]
        h0[:, ~valid] = 0.0
        x_t = np.ascontiguousarray(
            xT.reshape(KO, 128, W).transpose(1, 0, 2))
        h0_t = np.ascontiguousarray(
            h0.reshape(KO, 128, W).transpose(1, 0, 2))

        cosb = np.empty((128, W), np.float32)
        sinb = np.empty((128, W), np.float32)
        cosb[0:64] = cos_full[colsc].T
        cosb[64:128] = cos_full[colsc].T
        sinb[0:64] = sin_full[colsc].T
        sinb[64:128] = sin_full[colsc].T

        cmi = np.zeros((128, 2), np.uint8)
        if not valid[A0 - 1]:
            cmi[:, 0] = 1
        if not valid[A1]:
            cmi[:, 1] = 1

        in_maps.append({
            "hT0": h0_t, "x": x_t, "cosb": cosb, "sinb": sinb, "cmi": cmi,
            "wqkv": wqkv_all, "wo": wo_all, "wup": wup_all, "wdn": wdn_all,
            "cw": cw_all, "cb": cb_all, "lm": lm_shards[c],
            "sel2": sel2_host, "sel2T": sel2T_host,
            "idr": ident_f32, "idb": ident_bf16,
            "conf": conf_host, "conr": conr_host, "onr1": onr1_host,
            "onb": onb_host,
        })
    return in_maps


# ---------------------------------------------------------------------------
# entry point
# ---------------------------------------------------------------------------

def kernel(input_ids, embedding, init_hidden, Wqkv, Wo, Wup, conv_w, conv_b,
           Wdown, lm_head):
    global LAST_EXEC_NS
    from concourse.bass_utils import run_bass_kernel_spmd

    if "nc" not in _CACHED:
        _CACHED["nc"] = _build_nc(debug=DEBUG)
    nc = _CACHED["nc"]

    wkey = id(Wqkv)
    if _CACHED.get("wkey") != wkey:
        _CACHED["shared"] = _prep_shared(Wqkv, Wo, Wup, conv_w, conv_b,
                                         Wdown, lm_head)
        _CACHED["wkey"] = wkey

    in_maps = _prep_in_maps(input_ids, embedding, init_hidden,
                            _CACHED["shared"])

    res = run_bass_kernel_spmd(nc, in_maps, core_ids=list(range(NCORES)))
    if getattr(res, "exec_time_ns", None):
        LAST_EXEC_NS = res.exec_time_ns

    logits = np.concatenate(
        [np.asarray(res.results[c]["logits"]) for c in range(NCORES)], axis=1)
    return logits.reshape(B, N, V).astype(np.float32)
